# revision 20
# baseline (speedup 1.0000x reference)
"""Trainium2 Bass kernel for nn_AugmentedTensor (per-head bilinear form).

out[b,a,o] = sum_{i,j} h0[b,a,i] * h1[b,a,j] * T[a,i,j,o],  h = concat(x, 1)

Decomposition (i<128 from x0, i=128 is the ones row; same for j):
  main = sum_{i<128,j<128} x0[b,i] x1[b,j] T[a,i,j,o]
       -> stage1 (PE):  r[b,(j,o)] = x0_tile @ Tc[a]          (K=128, N=16384)
       -> stage2: z[b,j,o] = x1[b,j]*r[b,j,o] via wide VectorE tensor_tensor
          (in1 = fp16 x1 with stride-0-innermost AP, ~150 ns per 128-wide
          slice vs 342 for per-j STT) on 10/16 PSUM units per tile, and
          ScalarE activation-scale on the other 6; all 128 fp16 z slices
          summed by a 2x-mode halving tree whose ops run one tile late,
          interleaved into the next tile's unit loop.
  uv   = x0 @ T[a,:128,128,:] + x1 @ T[a,128,:128,:] + T[a,128,128,:]  (PE, small)

ACT and DVE units draw PSUM from separate pools so ScalarE's slow drain
(458 ns/slice) never blocks the PE->DVE fast path.

Sharding: 8 cores; core c -> head a=c>>1, batch half c&1 (2048 rows).
T traffic per core = one head (~8.5 MB) instead of 34 MB replicated.
"""

import numpy as np

BS, A, D, OUT = 4096, 4, 128, 128
NCORES = 8
BH = BS // 2      # batch rows per core
P = 128
NT = BH // P      # 16 tiles per core
NJ = 128          # j slices per tile
NCH = 32          # 512-wide stage-1 chunks per tile

# Stage-2 split by 1024-wide PSUM unit (8 j-slices each, 16 units/tile):
# units in ACT_UNITS go to ScalarE (8 per-partition-scale Copies into the
# fp16 zbuf); the rest are consumed by ONE wide VectorE tensor_tensor mult
# (PSUM f32 x stride-0-replicated fp16 h1 -> fp16 zbuf, 143 ns/slice vs 342
# for per-j STT). All 128 z slices are then tree-added in fp16 2x mode.
NU = 16  # 1024-wide units per tile
# evenly spaced including across the tile boundary (14 -> next tile's 1 is
# a 3-unit gap like the others), so ScalarE never idles a 4-unit stretch
ACT_UNITS = frozenset({1, 4, 6, 9, 11, 14})

_CACHE = {}
LAST_RESULT = None


def _split_sync_waits(bir_bytes):
    """The walrus build in this container supports exactly ONE sync-wait per
    instruction; Tile freely emits more. Hoist extra waits onto same-engine
    Nop instructions inserted immediately before the owner (engine streams
    are in-order, so 'wait then instruction' == 'instruction with wait').
    Extra completion-updates (non-DMA only) are hoisted onto following Nops.
    """
    import json

    bir = json.loads(bir_bytes)
    ctr = 0
    for fn in bir.get("functions", []):
        for blk in fn.get("blocks", []):
            ins_list = blk.get("instructions")
            if not ins_list:
                continue
            out = []
            for ins in ins_list:
                si = ins.get("sync_info")
                pre, post = [], []
                if si:
                    waits = si.get("on_wait") or []
                    if len(waits) > 1:
                        for w in waits[:-1]:
                            ctr += 1
                            pre.append({
                                "name": f"WSPLIT-{ctr}",
                                "opcode": "NoOp",
                                "engine": ins["engine"],
                                "debug": ins.get("debug", 0),
                                "ins": [],
                                "outs": [],
                                "sync_info": {"on_update": [], "on_wait": [w]},
                            })
                        si["on_wait"] = [waits[-1]]
                    ups = si.get("on_update") or []
                    if len(ups) > 1:
                        if ins.get("opcode") == "DMACopy":
                            raise RuntimeError(
                                f"DMACopy {ins['name']} has {len(ups)} updates; "
                                "cannot hoist safely")
                        for u in ups[1:]:
                            ctr += 1
                            post.append({
                                "name": f"USPLIT-{ctr}",
                                "opcode": "NoOp",
                                "engine": ins["engine"],
                                "debug": ins.get("debug", 0),
                                "ins": [],
                                "outs": [],
                                "sync_info": {"on_update": [u], "on_wait": []},
                            })
                        si["on_update"] = ups[:1]
                out.extend(pre)
                out.append(ins)
                out.extend(post)
            blk["instructions"] = out
    return json.dumps(bir).encode()


def _dedup_ldweights(bir_bytes):
    """Drop PE Ldweights whose operand payload matches the previous weight
    load with only Matmults in between (plain matmuls don't clobber the
    stationary array). Saves ~140 ns of PE time per reload; a tile's 32
    chunk-matmuls all share one x0 stationary. Sync info from a dropped
    load is merged onto the following instruction (the paired Matmult);
    _split_sync_waits then legalizes any multi-wait result.
    """
    import json

    bir = json.loads(bir_bytes)
    n_drop = 0
    for fn in bir.get("functions", []):
        for blk in fn.get("blocks", []):
            ins_list = blk.get("instructions")
            if not ins_list:
                continue
            last_load = None
            out = []
            drop_sync = None
            for ins in ins_list:
                if drop_sync is not None and ins.get("engine") == "PE":
                    si = ins.setdefault(
                        "sync_info", {"on_update": [], "on_wait": []})
                    si["on_wait"] = (drop_sync.get("on_wait") or []) + (
                        si.get("on_wait") or [])
                    si["on_update"] = (si.get("on_update") or []) + (
                        drop_sync.get("on_update") or [])
                    drop_sync = None
                if ins.get("opcode") == "Ldweights":
                    key = json.dumps(
                        [ins.get("ins"), ins.get("perf_mode"),
                         ins.get("is_transpose"), ins.get("tile_position")],
                        sort_keys=True)
                    if last_load == key:
                        si = ins.get("sync_info")
                        if si and (si.get("on_wait") or si.get("on_update")):
                            drop_sync = si
                        n_drop += 1
                        continue
                    last_load = key
                elif ins.get("engine") == "PE" and ins.get("opcode") != "Matmult":
                    last_load = None
                out.append(ins)
            assert drop_sync is None, "dropped Ldweights sync had no successor"
            blk["instructions"] = out
    return json.dumps(bir).encode()


def _install_compile_patch():
    """Route every BIR compile through the dedup + sync-split passes."""
    if _CACHE.get("patched"):
        return
    import concourse.bass_utils as bu

    orig = bu.compile_bir_kernel

    def patched(bir_json, tmpdir, neff_name="file.neff"):
        if isinstance(bir_json, str):
            bir_json = bir_json.encode()
        return orig(_split_sync_waits(_dedup_ldweights(bir_json)),
                    tmpdir, neff_name)

    bu.compile_bir_kernel = patched
    try:
        import concourse.bass2jax as b2j

        b2j.compile_bir_kernel = patched
    except ImportError:
        pass
    _CACHE["patched"] = True


def _build():
    import concourse.bass as bass
    import concourse.mybir as mybir
    from concourse.tile import TileContext

    f32 = mybir.dt.float32
    bf16 = mybir.dt.bfloat16
    f16 = mybir.dt.float16
    nc = bass.Bass()

    x0t = nc.dram_tensor("x0t", [P, BH], bf16, kind="ExternalInput")
    x1t = nc.dram_tensor("x1t", [P, BH], bf16, kind="ExternalInput")
    x1n = nc.dram_tensor("x1n", [BH, P], f32, kind="ExternalInput")
    tcm = nc.dram_tensor("tcm", [P, 128 * 128], bf16, kind="ExternalInput")
    tuv = nc.dram_tensor("tuv", [P, 256], bf16, kind="ExternalInput")
    tcc = nc.dram_tensor("tcc", [1, P], bf16, kind="ExternalInput")
    out = nc.dram_tensor("out", [BH, P], f32, kind="ExternalOutput")

    mult = mybir.AluOpType.mult
    add = mybir.AluOpType.add

    def grp3(ap, n_grp, inner):
        # [P, n_grp*inner] contiguous 2D AP -> 3D [P, n_grp, inner]
        return bass.AP(ap.tensor, ap.offset,
                       [list(ap.ap[0]), [inner, n_grp], [1, inner]])

    def rep_inner(ap, n_grp, n_rep):
        # [P, n_grp] 2D AP -> 3D [P, n_grp, n_rep] with innermost stride 0
        return bass.AP(ap.tensor, ap.offset,
                       [list(ap.ap[0]), [1, n_grp], [0, n_rep]])

    with TileContext(nc) as tc:
        with (
            tc.tile_pool(name="const", bufs=1) as cpool,
            tc.tile_pool(name="rpsum", bufs=2, space="PSUM") as ppool,
            tc.tile_pool(name="apsum", bufs=2, space="PSUM") as ppoolA,
            tc.tile_pool(name="acc", bufs=2) as apool,
        ):
            # --- resident constants (order = first-consumption order) ---
            x0t_s = cpool.tile([P, BH], bf16, tag="x0t")
            nc.sync.dma_start(x0t_s, x0t[:, :])
            x1t_s = cpool.tile([P, BH], bf16, tag="x1t")
            nc.sync.dma_start(x1t_s, x1t[:, :])
            tuv_s = cpool.tile([P, 256], bf16, tag="tuv")
            nc.sync.dma_start(tuv_s, tuv[:, :])
            tcc_s = cpool.tile([1, P], bf16, tag="tcc")
            nc.sync.dma_start(tcc_s, tcc[:, :])
            ones_s = cpool.tile([1, P], bf16, tag="ones")
            nc.vector.memset(ones_s, 1.0)
            x1n_all = cpool.tile([P, NT * P], f32, tag="x1n_all")
            for t in range(NT):
                nc.sync.dma_start(x1n_all[:, t * P : (t + 1) * P],
                                  x1n[t * P : (t + 1) * P, :])
            x1h_all = cpool.tile([P, NT * P], f16, tag="x1h_all")
            nc.vector.tensor_copy(x1h_all, x1n_all)
            tcm_tiles = []
            for g in range(16):
                tg = cpool.tile([P, 1024], bf16, tag=f"tcm{g}")
                nc.sync.dma_start(tg, tcm[:, g * 1024 : (g + 1) * 1024])
                tcm_tiles.append(tg)

            uvall = cpool.tile([P, NT * P], f32, tag="uvall")

            def make_finisher(zb, bsl_t):
                # closures for tile finalization: halving tree over 128
                # fp16 slices, then +bias-row terms and output DMA. Run
                # one tile late, interleaved into the next tile's unit
                # loop, so every dep is satisfied at issue time. Every op
                # is kept <= FD 2048 (~1.2 us) so a pop never delays the
                # next unit's tensor_tensor long enough to stall the PE
                # via PSUM WAR.
                ops = []

                def fold(dst, src, width):
                    def op():
                        nc.vector.tensor_add(
                            zb[:, dst : dst + width],
                            zb[:, dst : dst + width],
                            zb[:, src : src + width],
                        )
                    return op

                cur = NJ * P   # elements in the live prefix
                while cur > P:
                    half = cur // 2
                    for piece in range(0, half, 2048):
                        w = min(2048, half - piece)
                        ops.append(fold(piece, half + piece, w))
                    cur = half

                def finish():
                    out_t = apool.tile([P, P], f32, tag="out_t",
                                       name=f"out_{bsl_t.start}")
                    nc.vector.tensor_add(out_t, zb[:, :P], uvall[:, bsl_t])
                    nc.sync.dma_start(out[bsl_t, :], out_t)

                ops.append(finish)
                return ops

            pending = []
            for t in range(NT):
                bsl = slice(t * P, (t + 1) * P)
                x1n_t = x1n_all[:, bsl]
                x1h_t = x1h_all[:, bsl]

                # fp16 staging for all 128 z_j = x1[b,j]*r[b,j,:] slices
                zbuf = apool.tile([P, NJ * P], f16, tag="zbuf")

                # bias-row terms for this tile, just-in-time (keeps the PE
                # start-of-kernel free of a serial 48-matmul prelude). The
                # x0t stationary comes last so the unit loop's first
                # ldweights dedups against it.
                psm = ppool.tile([P, 1024], f32, tag="r", name=f"psm_{t}")
                nc.tensor.matmul(psm[:, 0:128], x1t_s[:, bsl],
                                 tuv_s[:, 128:256], start=True, stop=False)
                nc.tensor.matmul(psm[:, 0:128], ones_s, tcc_s,
                                 start=False, stop=False)
                nc.tensor.matmul(psm[:, 0:128], x0t_s[:, bsl],
                                 tuv_s[:, 0:128], start=False, stop=True)
                nc.vector.tensor_copy(uvall[:, bsl], psm[:, 0:128])

                for u in range(NU):
                    if pending:
                        pending.pop(0)()
                    pp = ppoolA if u in ACT_UNITS else ppool
                    r = pp.tile([P, 1024], f32, tag="r", name=f"r_{t}_{u}")
                    for q in range(2):
                        g = 2 * u + q
                        nc.tensor.matmul(
                            r[:, q * 512 : (q + 1) * 512],
                            x0t_s[:, bsl],
                            tcm_tiles[g // 2][:, (g % 2) * 512 : (g % 2) * 512 + 512],
                            start=True, stop=True,
                        )
                    j0 = 8 * u
                    if u in ACT_UNITS:
                        for jj in range(8):
                            nc.scalar.activation(
                                zbuf[:, (j0 + jj) * P : (j0 + jj + 1) * P],
                                r[:, jj * P : (jj + 1) * P],
                                mybir.ActivationFunctionType.Copy,
                                scale=x1n_t[:, j0 + jj : j0 + jj + 1],
                            )
                    else:
                        nc.vector.tensor_tensor(
                            grp3(zbuf[:, j0 * P : (j0 + 8) * P], 8, P),
                            grp3(r[:, :], 8, P),
                            rep_inner(x1h_t[:, j0 : j0 + 8], 8, P),
                            mult,
                        )

                pending.extend(make_finisher(zbuf, bsl))

            for op in pending:
                op()

    return nc


def _get_nc():
    if "nc" not in _CACHE:
        _CACHE["nc"] = _build()
    return _CACHE["nc"]


def _make_runner(nc):
    """Persistent sharded-jit runner for the axon/PJRT path (specialized copy
    of bass2jax.run_bass_via_pjrt so repeated calls reuse one compiled
    executable). Returns run(in_maps) -> list[dict[str, np.ndarray]]."""
    import jax
    import numpy as jnp_np  # noqa
    from jax.sharding import Mesh, PartitionSpec
    from jax.experimental.shard_map import shard_map
    import concourse.mybir as mybir
    from concourse.bass2jax import (
        _bass_exec_p, install_neuronx_cc_hook, partition_id_tensor)

    install_neuronx_cc_hook()

    partition_name = nc.partition_id_tensor.name if nc.partition_id_tensor else None
    in_names, out_names, out_avals, zero_outs = [], [], [], []
    for alloc in nc.m.functions[0].allocations:
        if not isinstance(alloc, mybir.MemoryLocationSet):
            continue
        name = alloc.memorylocations[0].name
        if alloc.kind == "ExternalInput":
            if name != partition_name:
                in_names.append(name)
        elif alloc.kind == "ExternalOutput":
            out_names.append(name)
            shape = tuple(alloc.tensor_shape)
            dtype = mybir.dt.np(alloc.dtype)
            out_avals.append(jax.core.ShapedArray(shape, dtype))
            zero_outs.append(np.zeros(shape, dtype))
    n_params = len(in_names)
    n_outs = len(out_avals)
    all_in_names = list(in_names) + list(out_names)
    if partition_name is not None:
        all_in_names.append(partition_name)
    donate = tuple(range(n_params, n_params + n_outs))

    def _body(*args):
        operands = list(args)
        if partition_name is not None:
            operands.append(partition_id_tensor())
        outs = _bass_exec_p.bind(
            *operands,
            out_avals=tuple(out_avals),
            in_names=tuple(all_in_names),
            out_names=tuple(out_names),
            lowering_input_output_aliases=(),
            sim_require_finite=True,
            sim_require_nnan=True,
            nc=nc,
        )
        return tuple(outs)

    devices = jax.devices()[:NCORES]
    mesh = Mesh(np.asarray(devices), ("core",))
    in_specs = (PartitionSpec("core"),) * (n_params + n_outs)
    out_specs = (PartitionSpec("core"),) * len(out_names)
    sharded = jax.jit(
        shard_map(_body, mesh=mesh, in_specs=in_specs, out_specs=out_specs,
                  check_rep=False),
        donate_argnums=donate, keep_unused=True)

    def run(in_maps, raw=False):
        concat_in = [
            np.concatenate([np.asarray(m[name]) for m in in_maps], axis=0)
            for name in in_names
        ]
        concat_zeros = [
            np.zeros((NCORES * z.shape[0], *z.shape[1:]), z.dtype)
            for z in zero_outs
        ]
        out_arrs = sharded(*concat_in, *concat_zeros)
        if raw:
            return out_arrs
        return [
            {name: np.asarray(out_arrs[i]).reshape(NCORES, *out_avals[i].shape)[c]
             for i, name in enumerate(out_names)}
            for c in range(NCORES)
        ]

    return run


def _run(nc, in_maps):
    """Execute on 8 cores; under axon go through the persistent PJRT runner."""
    from concourse._compat import axon_active

    _install_compile_patch()

    if axon_active():
        if "runner" not in _CACHE:
            _CACHE["runner"] = _make_runner(nc)
        return _CACHE["runner"](in_maps), None

    from concourse.bass_utils import run_bass_kernel_spmd

    res = run_bass_kernel_spmd(nc, in_maps, core_ids=list(range(NCORES)))
    return res.results, res


def _make_in_maps(x0, x1, T):
    import ml_dtypes

    bf16 = ml_dtypes.bfloat16
    x0 = np.asarray(x0, dtype=np.float32)
    x1 = np.asarray(x1, dtype=np.float32)
    T = np.asarray(T, dtype=np.float32)

    in_maps = []
    for c in range(NCORES):
        a, h = divmod(c, 2)
        bsl = slice(h * BH, (h + 1) * BH)
        x0c = np.ascontiguousarray(x0[bsl, a, :])  # (BH, 128)
        x1c = np.ascontiguousarray(x1[bsl, a, :])
        in_maps.append({
            "x0t": np.ascontiguousarray(x0c.T).astype(bf16),
            "x1t": np.ascontiguousarray(x1c.T).astype(bf16),
            "x1n": x1c,
            "tcm": np.ascontiguousarray(
                T[a, :128, :128, :].reshape(128, 128 * 128)).astype(bf16),
            "tuv": np.ascontiguousarray(
                np.concatenate([T[a, :128, 128, :], T[a, 128, :128, :]],
                               axis=1)).astype(bf16),
            "tcc": np.ascontiguousarray(
                T[a, 128, 128, :].reshape(1, 128)).astype(bf16),
        })
    return in_maps


def kernel(x0, x1, T):
    global LAST_RESULT

    in_maps = _make_in_maps(x0, x1, T)
    nc = _get_nc()
    results, LAST_RESULT = _run(nc, in_maps)

    full = np.empty((BS, A, OUT), dtype=np.float32)
    for c in range(NCORES):
        a, h = divmod(c, 2)
        full[h * BH : (h + 1) * BH, a, :] = results[c]["out"]
    return full



# revision 22
# speedup vs baseline: 1.0425x; 1.0425x over previous
"""Trainium2 Bass kernel for nn_AugmentedTensor (per-head bilinear form).

out[b,a,o] = sum_{i,j} h0[b,a,i] * h1[b,a,j] * T[a,i,j,o],  h = concat(x, 1)

Decomposition (i<128 from x0, i=128 is the ones row; same for j):
  main = sum_{i<128,j<128} x0[b,i] x1[b,j] T[a,i,j,o]
       -> stage1 (PE):  r[b,(j,o)] = x0_tile @ Tc[a]          (K=128, N=16384)
       -> stage2: z[b,j,o] = x1[b,j]*r[b,j,o] via wide VectorE tensor_tensor
          (in1 = fp16 x1 with stride-0-innermost AP, ~150 ns per 128-wide
          slice vs 342 for per-j STT) on 10/16 PSUM units per tile, and
          ScalarE activation-scale on the other 6; all 128 fp16 z slices
          summed by a 2x-mode halving tree whose ops run one tile late,
          interleaved into the next tile's unit loop.
  uv   = x0 @ T[a,:128,128,:] + x1 @ T[a,128,:128,:] + T[a,128,128,:]  (PE, small)

ACT and DVE units draw PSUM from separate pools so ScalarE's slow drain
(458 ns/slice) never blocks the PE->DVE fast path.

Sharding: 8 cores; core c -> head a=c>>1, batch half c&1 (2048 rows).
T traffic per core = one head (~8.5 MB) instead of 34 MB replicated.
"""

import numpy as np

BS, A, D, OUT = 4096, 4, 128, 128
NCORES = 8
BH = BS // 2      # batch rows per core
P = 128
NT = BH // P      # 16 tiles per core
NJ = 128          # j slices per tile
NCH = 32          # 512-wide stage-1 chunks per tile

# Stage-2 split by 1024-wide PSUM unit (8 j-slices each, 16 units/tile):
# units in ACT_UNITS go to ScalarE (8 per-partition-scale Copies into the
# fp16 zbuf); the rest are consumed by ONE wide VectorE tensor_tensor mult
# (PSUM f32 x stride-0-replicated fp16 h1 -> fp16 zbuf, 143 ns/slice vs 342
# for per-j STT). All 128 z slices are then tree-added in fp16 2x mode.
NU = 16  # 1024-wide units per tile
# evenly spaced including across the tile boundary (14 -> next tile's 1 is
# a 3-unit gap like the others), so ScalarE never idles a 4-unit stretch
ACT_UNITS = frozenset({1, 4, 6, 9, 11, 14})

_CACHE = {}
LAST_RESULT = None


def _split_sync_waits(bir_bytes):
    """The walrus build in this container supports exactly ONE sync-wait per
    instruction; Tile freely emits more. Hoist extra waits onto same-engine
    Nop instructions inserted immediately before the owner (engine streams
    are in-order, so 'wait then instruction' == 'instruction with wait').
    Extra completion-updates (non-DMA only) are hoisted onto following Nops.
    """
    import json

    bir = json.loads(bir_bytes)
    ctr = 0
    for fn in bir.get("functions", []):
        for blk in fn.get("blocks", []):
            ins_list = blk.get("instructions")
            if not ins_list:
                continue
            out = []
            for ins in ins_list:
                si = ins.get("sync_info")
                pre, post = [], []
                if si:
                    waits = si.get("on_wait") or []
                    if len(waits) > 1:
                        for w in waits[:-1]:
                            ctr += 1
                            pre.append({
                                "name": f"WSPLIT-{ctr}",
                                "opcode": "NoOp",
                                "engine": ins["engine"],
                                "debug": ins.get("debug", 0),
                                "ins": [],
                                "outs": [],
                                "sync_info": {"on_update": [], "on_wait": [w]},
                            })
                        si["on_wait"] = [waits[-1]]
                    ups = si.get("on_update") or []
                    if len(ups) > 1:
                        if ins.get("opcode") == "DMACopy":
                            raise RuntimeError(
                                f"DMACopy {ins['name']} has {len(ups)} updates; "
                                "cannot hoist safely")
                        for u in ups[1:]:
                            ctr += 1
                            post.append({
                                "name": f"USPLIT-{ctr}",
                                "opcode": "NoOp",
                                "engine": ins["engine"],
                                "debug": ins.get("debug", 0),
                                "ins": [],
                                "outs": [],
                                "sync_info": {"on_update": [u], "on_wait": []},
                            })
                        si["on_update"] = ups[:1]
                out.extend(pre)
                out.append(ins)
                out.extend(post)
            blk["instructions"] = out
    return json.dumps(bir).encode()


def _dedup_ldweights(bir_bytes):
    """Drop PE Ldweights whose operand payload matches the previous weight
    load with only Matmults in between (plain matmuls don't clobber the
    stationary array). Saves ~140 ns of PE time per reload; a tile's 32
    chunk-matmuls all share one x0 stationary. Sync info from a dropped
    load is merged onto the following instruction (the paired Matmult);
    _split_sync_waits then legalizes any multi-wait result.
    """
    import json

    bir = json.loads(bir_bytes)
    n_drop = 0
    for fn in bir.get("functions", []):
        for blk in fn.get("blocks", []):
            ins_list = blk.get("instructions")
            if not ins_list:
                continue
            last_load = None
            out = []
            drop_sync = None
            for ins in ins_list:
                if drop_sync is not None and ins.get("engine") == "PE":
                    si = ins.setdefault(
                        "sync_info", {"on_update": [], "on_wait": []})
                    si["on_wait"] = (drop_sync.get("on_wait") or []) + (
                        si.get("on_wait") or [])
                    si["on_update"] = (si.get("on_update") or []) + (
                        drop_sync.get("on_update") or [])
                    drop_sync = None
                if ins.get("opcode") == "Ldweights":
                    key = json.dumps(
                        [ins.get("ins"), ins.get("perf_mode"),
                         ins.get("is_transpose"), ins.get("tile_position")],
                        sort_keys=True)
                    if last_load == key:
                        si = ins.get("sync_info")
                        if si and (si.get("on_wait") or si.get("on_update")):
                            drop_sync = si
                        n_drop += 1
                        continue
                    last_load = key
                elif ins.get("engine") == "PE" and ins.get("opcode") != "Matmult":
                    last_load = None
                out.append(ins)
            assert drop_sync is None, "dropped Ldweights sync had no successor"
            blk["instructions"] = out
    return json.dumps(bir).encode()


def _install_compile_patch():
    """Route every BIR compile through the dedup + sync-split passes."""
    if _CACHE.get("patched"):
        return
    import concourse.bass_utils as bu

    orig = bu.compile_bir_kernel

    def patched(bir_json, tmpdir, neff_name="file.neff"):
        if isinstance(bir_json, str):
            bir_json = bir_json.encode()
        return orig(_split_sync_waits(_dedup_ldweights(bir_json)),
                    tmpdir, neff_name)

    bu.compile_bir_kernel = patched
    try:
        import concourse.bass2jax as b2j

        b2j.compile_bir_kernel = patched
    except ImportError:
        pass
    _CACHE["patched"] = True


def _build():
    import concourse.bass as bass
    import concourse.mybir as mybir
    from concourse.tile import TileContext

    f32 = mybir.dt.float32
    bf16 = mybir.dt.bfloat16
    f16 = mybir.dt.float16
    nc = bass.Bass()

    x0t = nc.dram_tensor("x0t", [P, BH], bf16, kind="ExternalInput")
    x1t = nc.dram_tensor("x1t", [P, BH], bf16, kind="ExternalInput")
    x1n = nc.dram_tensor("x1n", [BH, P], f32, kind="ExternalInput")
    tcm = nc.dram_tensor("tcm", [P, 128 * 128], bf16, kind="ExternalInput")
    tuv = nc.dram_tensor("tuv", [P, 256], bf16, kind="ExternalInput")
    tcc = nc.dram_tensor("tcc", [1, P], bf16, kind="ExternalInput")
    out = nc.dram_tensor("out", [BH, P], f32, kind="ExternalOutput")

    mult = mybir.AluOpType.mult
    add = mybir.AluOpType.add

    def grp3(ap, n_grp, inner):
        # [P, n_grp*inner] contiguous 2D AP -> 3D [P, n_grp, inner]
        return bass.AP(ap.tensor, ap.offset,
                       [list(ap.ap[0]), [inner, n_grp], [1, inner]])

    def rep_inner(ap, n_grp, n_rep):
        # [P, n_grp] 2D AP -> 3D [P, n_grp, n_rep] with innermost stride 0
        return bass.AP(ap.tensor, ap.offset,
                       [list(ap.ap[0]), [1, n_grp], [0, n_rep]])

    with TileContext(nc) as tc:
        with (
            tc.tile_pool(name="const", bufs=1) as cpool,
            tc.tile_pool(name="rpsum", bufs=2, space="PSUM") as ppool,
            tc.tile_pool(name="apsum", bufs=2, space="PSUM") as ppoolA,
            tc.tile_pool(name="acc", bufs=2) as apool,
        ):
            # --- resident constants (order = first-consumption order) ---
            x0t_s = cpool.tile([P, BH], bf16, tag="x0t")
            nc.sync.dma_start(x0t_s, x0t[:, :])
            x1t_s = cpool.tile([P, BH], bf16, tag="x1t")
            nc.sync.dma_start(x1t_s, x1t[:, :])
            tuv_s = cpool.tile([P, 256], bf16, tag="tuv")
            nc.sync.dma_start(tuv_s, tuv[:, :])
            tcc_s = cpool.tile([1, P], bf16, tag="tcc")
            nc.sync.dma_start(tcc_s, tcc[:, :])
            ones_s = cpool.tile([1, P], bf16, tag="ones")
            nc.vector.memset(ones_s, 1.0)
            x1n_all = cpool.tile([P, NT * P], f32, tag="x1n_all")
            for t in range(NT):
                nc.sync.dma_start(x1n_all[:, t * P : (t + 1) * P],
                                  x1n[t * P : (t + 1) * P, :])
            x1h_all = cpool.tile([P, NT * P], f16, tag="x1h_all")
            nc.vector.tensor_copy(x1h_all, x1n_all)
            tcm_tiles = []
            for g in range(16):
                tg = cpool.tile([P, 1024], bf16, tag=f"tcm{g}")
                nc.sync.dma_start(tg, tcm[:, g * 1024 : (g + 1) * 1024])
                tcm_tiles.append(tg)

            # prelude: bias-row terms for every tile -> SBUF, frees PSUM for r
            uvall = cpool.tile([P, NT * P], f32, tag="uvall")
            for t in range(NT):
                bsl = slice(t * P, (t + 1) * P)
                psm = ppool.tile([P, P], f32, tag="r", name=f"psm_{t}")
                nc.tensor.matmul(psm, x0t_s[:, bsl], tuv_s[:, 0:128],
                                 start=True, stop=False)
                nc.tensor.matmul(psm, x1t_s[:, bsl], tuv_s[:, 128:256],
                                 start=False, stop=False)
                nc.tensor.matmul(psm, ones_s, tcc_s, start=False, stop=True)
                nc.vector.tensor_copy(uvall[:, bsl], psm)

            def make_finisher(zb, bsl_t):
                # closures for tile finalization: halving tree over 128
                # fp16 slices, then +bias-row terms and output DMA. Run
                # one tile late, interleaved into the next tile's unit
                # loop, so every dep is satisfied at issue time. Every op
                # is kept <= FD 2048 (~1.2 us) so a pop never delays the
                # next unit's tensor_tensor long enough to stall the PE
                # via PSUM WAR.
                ops = []

                def fold(dst, src, width):
                    def op():
                        nc.vector.tensor_add(
                            zb[:, dst : dst + width],
                            zb[:, dst : dst + width],
                            zb[:, src : src + width],
                        )
                    return op

                cur = NJ * P   # elements in the live prefix
                while cur > P:
                    half = cur // 2
                    for piece in range(0, half, 2048):
                        w = min(2048, half - piece)
                        ops.append(fold(piece, half + piece, w))
                    cur = half

                def finish():
                    out_t = apool.tile([P, P], f32, tag="out_t",
                                       name=f"out_{bsl_t.start}")
                    nc.vector.tensor_add(out_t, zb[:, :P], uvall[:, bsl_t])
                    nc.sync.dma_start(out[bsl_t, :], out_t)

                ops.append(finish)
                return ops

            pending = []
            for t in range(NT):
                bsl = slice(t * P, (t + 1) * P)
                x1n_t = x1n_all[:, bsl]
                x1h_t = x1h_all[:, bsl]

                # fp16 staging for all 128 z_j = x1[b,j]*r[b,j,:] slices
                zbuf = apool.tile([P, NJ * P], f16, tag="zbuf")

                for u in range(NU):
                    if pending:
                        pending.pop(0)()
                    pp = ppoolA if u in ACT_UNITS else ppool
                    r = pp.tile([P, 1024], f32, tag="r", name=f"r_{t}_{u}")
                    for q in range(2):
                        g = 2 * u + q
                        nc.tensor.matmul(
                            r[:, q * 512 : (q + 1) * 512],
                            x0t_s[:, bsl],
                            tcm_tiles[g // 2][:, (g % 2) * 512 : (g % 2) * 512 + 512],
                            start=True, stop=True,
                        )
                    j0 = 8 * u
                    if u in ACT_UNITS:
                        for jj in range(8):
                            nc.scalar.activation(
                                zbuf[:, (j0 + jj) * P : (j0 + jj + 1) * P],
                                r[:, jj * P : (jj + 1) * P],
                                mybir.ActivationFunctionType.Copy,
                                scale=x1n_t[:, j0 + jj : j0 + jj + 1],
                            )
                    else:
                        nc.vector.tensor_tensor(
                            grp3(zbuf[:, j0 * P : (j0 + 8) * P], 8, P),
                            grp3(r[:, :], 8, P),
                            rep_inner(x1h_t[:, j0 : j0 + 8], 8, P),
                            mult,
                        )

                pending.extend(make_finisher(zbuf, bsl))

            for op in pending:
                op()

    return nc


def _get_nc():
    if "nc" not in _CACHE:
        _CACHE["nc"] = _build()
    return _CACHE["nc"]


def _make_runner(nc):
    """Persistent sharded-jit runner for the axon/PJRT path (specialized copy
    of bass2jax.run_bass_via_pjrt so repeated calls reuse one compiled
    executable). Returns run(in_maps) -> list[dict[str, np.ndarray]]."""
    import jax
    import numpy as jnp_np  # noqa
    from jax.sharding import Mesh, PartitionSpec
    from jax.experimental.shard_map import shard_map
    import concourse.mybir as mybir
    from concourse.bass2jax import (
        _bass_exec_p, install_neuronx_cc_hook, partition_id_tensor)

    install_neuronx_cc_hook()

    partition_name = nc.partition_id_tensor.name if nc.partition_id_tensor else None
    in_names, out_names, out_avals, zero_outs = [], [], [], []
    for alloc in nc.m.functions[0].allocations:
        if not isinstance(alloc, mybir.MemoryLocationSet):
            continue
        name = alloc.memorylocations[0].name
        if alloc.kind == "ExternalInput":
            if name != partition_name:
                in_names.append(name)
        elif alloc.kind == "ExternalOutput":
            out_names.append(name)
            shape = tuple(alloc.tensor_shape)
            dtype = mybir.dt.np(alloc.dtype)
            out_avals.append(jax.core.ShapedArray(shape, dtype))
            zero_outs.append(np.zeros(shape, dtype))
    n_params = len(in_names)
    n_outs = len(out_avals)
    all_in_names = list(in_names) + list(out_names)
    if partition_name is not None:
        all_in_names.append(partition_name)
    donate = tuple(range(n_params, n_params + n_outs))

    def _body(*args):
        operands = list(args)
        if partition_name is not None:
            operands.append(partition_id_tensor())
        outs = _bass_exec_p.bind(
            *operands,
            out_avals=tuple(out_avals),
            in_names=tuple(all_in_names),
            out_names=tuple(out_names),
            lowering_input_output_aliases=(),
            sim_require_finite=True,
            sim_require_nnan=True,
            nc=nc,
        )
        return tuple(outs)

    devices = jax.devices()[:NCORES]
    mesh = Mesh(np.asarray(devices), ("core",))
    in_specs = (PartitionSpec("core"),) * (n_params + n_outs)
    out_specs = (PartitionSpec("core"),) * len(out_names)
    sharded = jax.jit(
        shard_map(_body, mesh=mesh, in_specs=in_specs, out_specs=out_specs,
                  check_rep=False),
        donate_argnums=donate, keep_unused=True)

    def run(in_maps, raw=False):
        concat_in = [
            np.concatenate([np.asarray(m[name]) for m in in_maps], axis=0)
            for name in in_names
        ]
        concat_zeros = [
            np.zeros((NCORES * z.shape[0], *z.shape[1:]), z.dtype)
            for z in zero_outs
        ]
        out_arrs = sharded(*concat_in, *concat_zeros)
        if raw:
            return out_arrs
        return [
            {name: np.asarray(out_arrs[i]).reshape(NCORES, *out_avals[i].shape)[c]
             for i, name in enumerate(out_names)}
            for c in range(NCORES)
        ]

    return run


def _run(nc, in_maps):
    """Execute on 8 cores; under axon go through the persistent PJRT runner."""
    from concourse._compat import axon_active

    _install_compile_patch()

    if axon_active():
        if "runner" not in _CACHE:
            _CACHE["runner"] = _make_runner(nc)
        return _CACHE["runner"](in_maps), None

    from concourse.bass_utils import run_bass_kernel_spmd

    res = run_bass_kernel_spmd(nc, in_maps, core_ids=list(range(NCORES)))
    return res.results, res


def _make_in_maps(x0, x1, T):
    import ml_dtypes

    bf16 = ml_dtypes.bfloat16
    x0 = np.asarray(x0, dtype=np.float32)
    x1 = np.asarray(x1, dtype=np.float32)
    T = np.asarray(T, dtype=np.float32)

    in_maps = []
    for c in range(NCORES):
        a, h = divmod(c, 2)
        bsl = slice(h * BH, (h + 1) * BH)
        x0c = np.ascontiguousarray(x0[bsl, a, :])  # (BH, 128)
        x1c = np.ascontiguousarray(x1[bsl, a, :])
        in_maps.append({
            "x0t": np.ascontiguousarray(x0c.T).astype(bf16),
            "x1t": np.ascontiguousarray(x1c.T).astype(bf16),
            "x1n": x1c,
            "tcm": np.ascontiguousarray(
                T[a, :128, :128, :].reshape(128, 128 * 128)).astype(bf16),
            "tuv": np.ascontiguousarray(
                np.concatenate([T[a, :128, 128, :], T[a, 128, :128, :]],
                               axis=1)).astype(bf16),
            "tcc": np.ascontiguousarray(
                T[a, 128, 128, :].reshape(1, 128)).astype(bf16),
        })
    return in_maps


def kernel(x0, x1, T):
    global LAST_RESULT

    in_maps = _make_in_maps(x0, x1, T)
    nc = _get_nc()
    results, LAST_RESULT = _run(nc, in_maps)

    full = np.empty((BS, A, OUT), dtype=np.float32)
    for c in range(NCORES):
        a, h = divmod(c, 2)
        full[h * BH : (h + 1) * BH, a, :] = results[c]["out"]
    return full

